# revision 24
# baseline (speedup 1.0000x reference)
"""CrossAttention kernel for 8 Trainium2 NeuronCores.

Problem (hardcoded): B=8, SQ=SK=1024, Q_DIM=2048, KV_DIM=1024, E_DIM=2048,
H=16 heads, HD=128.  out = softmax((X_q Wq^T + bq)(X_k Wk^T + bk)^T / sqrt(HD))
                            @ (X_v Wv^T + bv) @ Wo^T + bo

Sharding: data-parallel over batch — each of the 8 cores computes one batch
element end-to-end; no collectives.

Per-core dataflow (all matmuls bf16 with f32 PSUM accumulation):
  - Host pre-transposes activations/weights so every matmul sees natural
    [K_contraction on partitions] layouts:
      qT[e,s]  = (WqT tiles).T @ xqT      (e on partitions, per-head blocks)
      kT[e,s], vT[e,s] likewise; vT is DMA-transposed into v[s, e] with a
      ones column appended per head (gives softmax denominators for free).
  - Per head: scoresT[sk,sq] = kT_h.T @ qT_h ; P = exp(scoresT) (ACT, bf16)
      oP[sq, 129] = sum_sk P_tile.T @ v[sk, head||ones]   (col 128 = rowsum)
      ao[s, e] = oP[:, :128] * (1/oP[:, 128])             (per-partition)
  - ao is DMA-transposed to aoT[e, s]; out = aoT.T @ WoT (+ bo on host).
"""

import sys

sys.path.insert(0, "/opt/trn_rl_repo")

import numpy as np
import ml_dtypes

import concourse.tile as tile
from concourse import bacc
import concourse.mybir as mybir
from concourse.bass_utils import run_bass_kernel_spmd

F32 = mybir.dt.float32
BF16 = mybir.dt.bfloat16
ACT_IDENT = mybir.ActivationFunctionType.Identity
ACT_EXP = mybir.ActivationFunctionType.Exp

B = 8
S = 1024          # SQ == SK
DQ = 2048         # query input dim
DKV = 1024        # key/value input dim
E = 2048          # embed dim
H = 16            # heads
HD = 128          # head dim
NT_S = S // 128   # 8 seq tiles
NT_E = E // 128   # 16 e tiles (== heads)
NT_DQ = DQ // 128
NT_DKV = DKV // 128
VROW = HD + 1     # head block in v_sb incl. ones column

_CACHED = {}


def _build():
    nc = bacc.Bacc("TRN2", target_bir_lowering=False, debug=False)

    xqT = nc.dram_tensor("xqT", [DQ, S], BF16, kind="ExternalInput")
    xkT = nc.dram_tensor("xkT", [DKV, S], BF16, kind="ExternalInput")
    xvT = nc.dram_tensor("xvT", [DKV, S], BF16, kind="ExternalInput")
    wqT = nc.dram_tensor("wqT", [DQ, E], BF16, kind="ExternalInput")
    wkT = nc.dram_tensor("wkT", [DKV, E], BF16, kind="ExternalInput")
    wvT = nc.dram_tensor("wvT", [DKV, E], BF16, kind="ExternalInput")
    woT = nc.dram_tensor("woT", [E, E], BF16, kind="ExternalInput")
    bq = nc.dram_tensor("bq", [E], F32, kind="ExternalInput")
    bk = nc.dram_tensor("bk", [E], F32, kind="ExternalInput")
    bv = nc.dram_tensor("bv", [E], F32, kind="ExternalInput")
    out = nc.dram_tensor("out", [S, E], F32, kind="ExternalOutput")

    with tile.TileContext(nc) as tc:
        with (
            tc.tile_pool(name="persist", bufs=1) as persist,
            tc.tile_pool(name="wstream", bufs=6) as wstream,
            tc.tile_pool(name="wostream", bufs=4) as wostream,
            tc.tile_pool(name="pts", bufs=15) as pts,
            tc.tile_pool(name="outsb", bufs=1) as outsb,
            tc.tile_pool(name="small", bufs=2) as small,
            tc.tile_pool(name="bigps", bufs=4, space="PSUM") as bigps,
        ):
            # ---- load inputs / constants ----
            xq_sb = persist.tile([128, NT_DQ, S], BF16, tag="big_a")
            xk_sb = [pts.tile([128, S], BF16, tag="pt", name=f"xk{_d}")
                     for _d in range(NT_DKV)]
            xv_sb = [pts.tile([128, S], BF16, tag="pt", name=f"xv{_d}")
                     for _d in range(NT_DKV)]
            xqT_r = xqT.rearrange("(t p) s -> p t s", p=128)
            for _d in range(NT_DQ):
                nc.sync.dma_start(out=xq_sb[:, _d, :], in_=xqT_r[:, _d, :])
            xkT_r = xkT.rearrange("(t p) s -> p t s", p=128)
            xvT_r = xvT.rearrange("(t p) s -> p t s", p=128)
            for _d in range(NT_DKV):
                nc.sync.dma_start(out=xk_sb[_d], in_=xkT_r[:, _d, :])
                nc.sync.dma_start(out=xv_sb[_d], in_=xvT_r[:, _d, :])
            bq_sb = persist.tile([128, NT_E], F32, tag="bq")
            bk_sb = persist.tile([128, NT_E], F32, tag="bk")
            bv_sb = persist.tile([128, NT_E], F32, tag="bv")
            nc.sync.dma_start(out=bq_sb, in_=bq.rearrange("(t p) -> p t", p=128))
            nc.sync.dma_start(out=bk_sb, in_=bk.rearrange("(t p) -> p t", p=128))
            nc.sync.dma_start(out=bv_sb, in_=bv.rearrange("(t p) -> p t", p=128))

            qT_sb = persist.tile([128, NT_E, S], BF16, tag="qT")
            kT_sb = persist.tile([128, NT_E, S], BF16, tag="kT")
            vT_sb = persist.tile([128, NT_E, S], BF16, tag="big_b")
            v_sb = persist.tile([128, NT_S, H * VROW], BF16, tag="v")
            # ones columns for the softmax-denominator trick
            nc.vector.memset(
                v_sb.rearrange("p t (h c) -> p t h c", c=VROW)[:, :, :, HD:], 1.0
            )

            # ---- projections: produce qT/kT/vT in [e_partition, s] layout ----
            # out_psum[e128, s1024] accumulated over d tiles; weight blocks of
            # 256 e-columns streamed once; X resident.
            def project(w_dram, x_sb, nt_d, bias_sb, dst_sb, wtag):
                for eb in range(E // 256):
                    ps = [bigps.tile([128, S], F32, tag="ps", name=f"projps{_i}")
                          for _i in range(2)]
                    for d in range(nt_d):
                        wt = wstream.tile([128, 256], BF16, tag="w",
                                          name=f"w_{wtag}_{eb}_{d}")
                        nc.scalar.dma_start(
                            out=wt,
                            in_=w_dram[d * 128:(d + 1) * 128, eb * 256:(eb + 1) * 256],
                        )
                        xd = x_sb[:, d, :] if not isinstance(x_sb, list) \
                            else x_sb[d]
                        for es in range(2):
                            for sc in range(2):
                                nc.tensor.matmul(
                                    ps[es][:, sc * 512:(sc + 1) * 512],
                                    wt[:, es * 128:(es + 1) * 128],
                                    xd[:, sc * 512:(sc + 1) * 512],
                                    start=(d == 0),
                                    stop=(d == nt_d - 1),
                                )
                    for es in range(2):
                        et = eb * 2 + es
                        nc.scalar.activation(
                            out=dst_sb[:, et, :],
                            in_=ps[es],
                            func=ACT_IDENT,
                            bias=bias_sb[:, et:et + 1],
                            scale=1.0,
                        )

            project(wqT, xq_sb, NT_DQ, bq_sb, qT_sb, "wq")
            project(wkT, xk_sb, NT_DKV, bk_sb, kT_sb, "wk")
            project(wvT, xv_sb, NT_DKV, bv_sb, vT_sb, "wv")

            # vT[e,s] -> v[s, head-block] via DMA transpose (per head/e-tile)
            for h in range(H):
                vtmp = wstream.tile([128, NT_S, HD], BF16, tag="vtmp",
                                    name=f"vtmp{h}", bufs=2)
                nc.sync.dma_start_transpose(out=vtmp, in_=vT_sb[:, h, :])
                nc.vector.tensor_copy(
                    out=v_sb[:, :, h * VROW:h * VROW + HD], in_=vtmp)

            # ---- attention per head ----
            ao_sb = persist.tile([128, NT_S, E], BF16, tag="big_a")
            for h in range(H):
                pt_t = [pts.tile([128, S], BF16, tag="pt", name=f"pt{h}_{_sk}")
                        for _sk in range(NT_S)]
                for sk in range(NT_S):
                    ssp = bigps.tile([128, S], F32, tag="ps")
                    for sc in range(2):
                        nc.tensor.matmul(
                            ssp[:, sc * 512:(sc + 1) * 512],
                            kT_sb[:, h, sk * 128:(sk + 1) * 128],
                            qT_sb[:, h, sc * 512:(sc + 1) * 512],
                            start=True,
                            stop=True,
                        )
                    nc.scalar.activation(
                        out=pt_t[sk], in_=ssp, func=ACT_EXP, bias=0.0, scale=1.0
                    )
                for sq in range(NT_S):
                    op_t = bigps.tile([128, S], F32, tag="ps", name=f"op{h}_{sq}")
                    op = op_t[:, 0:VROW]
                    for sk in range(NT_S):
                        nc.tensor.matmul(
                            op,
                            pt_t[sk][:, sq * 128:(sq + 1) * 128],
                            v_sb[:, sk, h * VROW:(h + 1) * VROW],
                            start=(sk == 0),
                            stop=(sk == NT_S - 1),
                        )
                    recip = small.tile([128, 1], F32, tag="recip")
                    nc.vector.reciprocal(out=recip, in_=op[:, HD:VROW])
                    nc.vector.tensor_scalar_mul(
                        ao_sb[:, sq, h * 128:(h + 1) * 128], op[:, 0:HD], recip
                    )

            # ---- transpose ao[s,e] -> aoT[e,s] ----
            aoT_sb = persist.tile([128, NT_E, S], BF16, tag="big_b")
            for sq in range(NT_S):
                nc.sync.dma_start_transpose(
                    out=aoT_sb[:, :, sq * 128:(sq + 1) * 128],
                    in_=ao_sb[:, sq, :],
                )

            # ---- output projection: out[s, e2] = aoT.T @ WoT ----
            for sg in range(2):
                for ep in range(2):  # 1024-wide e2 column halves
                    pso = [bigps.tile([128, S], F32, tag="ps", name=f"pso{_i}")
                           for _i in range(NT_S // 2)]
                    for et in range(NT_E):
                        wo_t = wostream.tile([128, 1024], BF16, tag="wo")
                        nc.scalar.dma_start(
                            out=wo_t,
                            in_=woT[et * 128:(et + 1) * 128,
                                    ep * 1024:(ep + 1) * 1024],
                        )
                        for st4 in range(NT_S // 2):
                            st = sg * (NT_S // 2) + st4
                            for ech in range(2):
                                nc.tensor.matmul(
                                    pso[st4][:, ech * 512:(ech + 1) * 512],
                                    aoT_sb[:, et, st * 128:(st + 1) * 128],
                                    wo_t[:, ech * 512:(ech + 1) * 512],
                                    start=(et == 0),
                                    stop=(et == NT_E - 1),
                                )
                    for st4 in range(NT_S // 2):
                        st = sg * (NT_S // 2) + st4
                        for ech in range(2):
                            ot = outsb.tile([128, 512], F32, tag="outt",
                                            name=f"ot{sg}_{ep}_{st4}_{ech}")
                            nc.vector.tensor_copy(
                                out=ot, in_=pso[st4][:, ech * 512:(ech + 1) * 512])
                            nc.sync.dma_start(
                                out=out.ap()[st * 128:(st + 1) * 128,
                                             (ep * 1024 + ech * 512):
                                             (ep * 1024 + (ech + 1) * 512)],
                                in_=ot,
                            )

    nc.compile()
    return nc


def _get_nc():
    if "nc" not in _CACHED:
        _CACHED["nc"] = _build()
    return _CACHED["nc"]


def _numpy_reference(query, key, value, attention_mask,
                     Wq, bq, Wk, bk, Wv, bv, Wo, bo):
    # general fallback (only used when attention_mask isn't all ones)
    Bb, SQ, _ = query.shape
    SK = key.shape[1]
    q = query @ Wq.T + bq
    k = key @ Wk.T + bk
    v = value @ Wv.T + bv
    q = q.reshape(Bb, SQ, H, HD).transpose(0, 2, 1, 3)
    k = k.reshape(Bb, SK, H, HD).transpose(0, 2, 1, 3)
    v = v.reshape(Bb, SK, H, HD).transpose(0, 2, 1, 3)
    scores = np.einsum("bhqd,bhkd->bhqk", q, k) * (HD ** -0.5)
    scores = np.where(attention_mask[:, None, :, :] == 0,
                      np.float32(-1e10), scores)
    scores -= scores.max(-1, keepdims=True)
    p = np.exp(scores)
    p /= p.sum(-1, keepdims=True)
    o = np.einsum("bhqk,bhkd->bhqd", p, v)
    o = o.transpose(0, 2, 1, 3).reshape(Bb, SQ, E)
    return (o @ Wo.T + bo).astype(np.float32)


def _prepare_in_maps(inputs):
    query = np.asarray(inputs["query"], dtype=np.float32)
    key = np.asarray(inputs["key"], dtype=np.float32)
    value = np.asarray(inputs["value"], dtype=np.float32)
    Wq = np.asarray(inputs["Wq"], dtype=np.float32)
    bq = np.asarray(inputs["bq"], dtype=np.float32)
    Wk = np.asarray(inputs["Wk"], dtype=np.float32)
    bk = np.asarray(inputs["bk"], dtype=np.float32)
    Wv = np.asarray(inputs["Wv"], dtype=np.float32)
    bv = np.asarray(inputs["bv"], dtype=np.float32)
    Wo = np.asarray(inputs["Wo"], dtype=np.float32)

    scale = np.float32(HD ** -0.5)
    bf = ml_dtypes.bfloat16
    wqT = np.ascontiguousarray((Wq.T * scale).astype(bf))
    wkT = np.ascontiguousarray(Wk.T.astype(bf))
    wvT = np.ascontiguousarray(Wv.T.astype(bf))
    woT = np.ascontiguousarray(Wo.T.astype(bf))
    bq_s = (bq * scale).astype(np.float32)

    in_maps = []
    for b in range(B):
        in_maps.append({
            "xqT": np.ascontiguousarray(query[b].T.astype(bf)),
            "xkT": np.ascontiguousarray(key[b].T.astype(bf)),
            "xvT": np.ascontiguousarray(value[b].T.astype(bf)),
            "wqT": wqT, "wkT": wkT, "wvT": wvT, "woT": woT,
            "bq": bq_s, "bk": bk.astype(np.float32),
            "bv": bv.astype(np.float32),
        })
    return in_maps


def run_on_device(inputs, **spmd_kwargs):
    """Run the bass kernel; returns (out [B,S,E] f32, BassKernelResults)."""
    in_maps = _prepare_in_maps(inputs)
    bo = np.asarray(inputs["bo"], dtype=np.float32)
    res = run_bass_kernel_spmd(_get_nc(), in_maps,
                               core_ids=list(range(B)), **spmd_kwargs)
    out = np.stack([res.results[b]["out"] for b in range(B)], axis=0)
    return (out + bo).astype(np.float32), res


def kernel(**inputs):
    mask = np.asarray(inputs["attention_mask"])
    if not mask.all():
        return _numpy_reference(
            np.asarray(inputs["query"], dtype=np.float32),
            np.asarray(inputs["key"], dtype=np.float32),
            np.asarray(inputs["value"], dtype=np.float32), mask,
            np.asarray(inputs["Wq"], dtype=np.float32),
            np.asarray(inputs["bq"], dtype=np.float32),
            np.asarray(inputs["Wk"], dtype=np.float32),
            np.asarray(inputs["bk"], dtype=np.float32),
            np.asarray(inputs["Wv"], dtype=np.float32),
            np.asarray(inputs["bv"], dtype=np.float32),
            np.asarray(inputs["Wo"], dtype=np.float32),
            np.asarray(inputs["bo"], dtype=np.float32))
    out, _ = run_on_device(inputs)
    return out


# revision 25
# speedup vs baseline: 1.0326x; 1.0326x over previous
"""CrossAttention kernel for 8 Trainium2 NeuronCores.

Problem (hardcoded): B=8, SQ=SK=1024, Q_DIM=2048, KV_DIM=1024, E_DIM=2048,
H=16 heads, HD=128.  out = softmax((X_q Wq^T + bq)(X_k Wk^T + bk)^T / sqrt(HD))
                            @ (X_v Wv^T + bv) @ Wo^T + bo

Sharding: data-parallel over batch — each of the 8 cores computes one batch
element end-to-end; no collectives.

Per-core dataflow (all matmuls bf16 with f32 PSUM accumulation):
  - Host pre-transposes activations/weights so every matmul sees natural
    [K_contraction on partitions] layouts:
      qT[e,s]  = (WqT tiles).T @ xqT      (e on partitions, per-head blocks)
      kT[e,s], vT[e,s] likewise; vT is DMA-transposed into v[s, e] with a
      ones column appended per head (gives softmax denominators for free).
  - Per head: scoresT[sk,sq] = kT_h.T @ qT_h ; P = exp(scoresT) (ACT, bf16)
      oP[sq, 129] = sum_sk P_tile.T @ v[sk, head||ones]   (col 128 = rowsum)
      ao[s, e] = oP[:, :128] * (1/oP[:, 128])             (per-partition)
  - ao is DMA-transposed to aoT[e, s]; out = aoT.T @ WoT (+ bo on host).
"""

import sys

sys.path.insert(0, "/opt/trn_rl_repo")

import numpy as np
import ml_dtypes

import concourse.tile as tile
from concourse import bacc
import concourse.mybir as mybir
from concourse.bass_utils import run_bass_kernel_spmd

F32 = mybir.dt.float32
BF16 = mybir.dt.bfloat16
ACT_IDENT = mybir.ActivationFunctionType.Identity
ACT_EXP = mybir.ActivationFunctionType.Exp

B = 8
S = 1024          # SQ == SK
DQ = 2048         # query input dim
DKV = 1024        # key/value input dim
E = 2048          # embed dim
H = 16            # heads
HD = 128          # head dim
NT_S = S // 128   # 8 seq tiles
NT_E = E // 128   # 16 e tiles (== heads)
NT_DQ = DQ // 128
NT_DKV = DKV // 128
VROW = HD + 1     # head block in v_sb incl. ones column

_CACHED = {}


def _build():
    nc = bacc.Bacc("TRN2", target_bir_lowering=False, debug=False)

    xqT = nc.dram_tensor("xqT", [DQ, S], BF16, kind="ExternalInput")
    xkT = nc.dram_tensor("xkT", [DKV, S], BF16, kind="ExternalInput")
    xvT = nc.dram_tensor("xvT", [DKV, S], BF16, kind="ExternalInput")
    wqT = nc.dram_tensor("wqT", [DQ, E], BF16, kind="ExternalInput")
    wkT = nc.dram_tensor("wkT", [DKV, E], BF16, kind="ExternalInput")
    wvT = nc.dram_tensor("wvT", [DKV, E], BF16, kind="ExternalInput")
    woT = nc.dram_tensor("woT", [E, E], BF16, kind="ExternalInput")
    bq = nc.dram_tensor("bq", [E], F32, kind="ExternalInput")
    bk = nc.dram_tensor("bk", [E], F32, kind="ExternalInput")
    bv = nc.dram_tensor("bv", [E], F32, kind="ExternalInput")
    out = nc.dram_tensor("out", [S, E], F32, kind="ExternalOutput")

    with tile.TileContext(nc) as tc:
        with (
            tc.tile_pool(name="persist", bufs=1) as persist,
            tc.tile_pool(name="wstream", bufs=6) as wstream,
            tc.tile_pool(name="pts", bufs=15) as pts,
            tc.tile_pool(name="outsb", bufs=1) as outsb,
            tc.tile_pool(name="small", bufs=2) as small,
            tc.tile_pool(name="bigps", bufs=4, space="PSUM") as bigps,
        ):
            # ---- load inputs / constants ----
            xq_sb = persist.tile([128, NT_DQ, S], BF16, tag="big_a")
            xk_sb = [pts.tile([128, S], BF16, tag="pt", name=f"xk{_d}")
                     for _d in range(NT_DKV)]
            xv_sb = [pts.tile([128, S], BF16, tag="pt", name=f"xv{_d}")
                     for _d in range(NT_DKV)]
            xqT_r = xqT.rearrange("(t p) s -> p t s", p=128)
            for _d in range(NT_DQ):
                nc.sync.dma_start(out=xq_sb[:, _d, :], in_=xqT_r[:, _d, :])
            xkT_r = xkT.rearrange("(t p) s -> p t s", p=128)
            xvT_r = xvT.rearrange("(t p) s -> p t s", p=128)
            for _d in range(NT_DKV):
                nc.sync.dma_start(out=xk_sb[_d], in_=xkT_r[:, _d, :])
                nc.sync.dma_start(out=xv_sb[_d], in_=xvT_r[:, _d, :])
            bq_sb = persist.tile([128, NT_E], F32, tag="bq")
            bk_sb = persist.tile([128, NT_E], F32, tag="bk")
            bv_sb = persist.tile([128, NT_E], F32, tag="bv")
            nc.sync.dma_start(out=bq_sb, in_=bq.rearrange("(t p) -> p t", p=128))
            nc.sync.dma_start(out=bk_sb, in_=bk.rearrange("(t p) -> p t", p=128))
            nc.sync.dma_start(out=bv_sb, in_=bv.rearrange("(t p) -> p t", p=128))

            qT_sb = persist.tile([128, NT_E, S], BF16, tag="qT")
            kT_sb = persist.tile([128, NT_E, S], BF16, tag="kT")
            vT_sb = persist.tile([128, NT_E, S], BF16, tag="big_b")
            v_sb = persist.tile([128, NT_S, H * VROW], BF16, tag="v")
            # ones columns for the softmax-denominator trick
            nc.vector.memset(
                v_sb.rearrange("p t (h c) -> p t h c", c=VROW)[:, :, :, HD:], 1.0
            )

            # ---- projections: produce qT/kT/vT in [e_partition, s] layout ----
            # out_psum[e128, s1024] accumulated over d tiles; weight blocks of
            # 256 e-columns streamed once; X resident.
            def project(w_dram, x_sb, nt_d, bias_sb, dst_sb, wtag):
                for eb in range(E // 256):
                    ps = [bigps.tile([128, S], F32, tag="ps", name=f"projps{_i}")
                          for _i in range(2)]
                    for d in range(nt_d):
                        wt = wstream.tile([128, 256], BF16, tag="w",
                                          name=f"w_{wtag}_{eb}_{d}")
                        nc.scalar.dma_start(
                            out=wt,
                            in_=w_dram[d * 128:(d + 1) * 128, eb * 256:(eb + 1) * 256],
                        )
                        xd = x_sb[:, d, :] if not isinstance(x_sb, list) \
                            else x_sb[d]
                        for es in range(2):
                            for sc in range(2):
                                nc.tensor.matmul(
                                    ps[es][:, sc * 512:(sc + 1) * 512],
                                    wt[:, es * 128:(es + 1) * 128],
                                    xd[:, sc * 512:(sc + 1) * 512],
                                    start=(d == 0),
                                    stop=(d == nt_d - 1),
                                )
                    for es in range(2):
                        et = eb * 2 + es
                        nc.scalar.activation(
                            out=dst_sb[:, et, :],
                            in_=ps[es],
                            func=ACT_IDENT,
                            bias=bias_sb[:, et:et + 1],
                            scale=1.0,
                        )

            project(wqT, xq_sb, NT_DQ, bq_sb, qT_sb, "wq")
            project(wkT, xk_sb, NT_DKV, bk_sb, kT_sb, "wk")
            project(wvT, xv_sb, NT_DKV, bv_sb, vT_sb, "wv")

            # vT[e,s] -> v[s, head-block] via DMA transpose (per head/e-tile)
            for h in range(H):
                vtmp = wstream.tile([128, NT_S, HD], BF16, tag="vtmp",
                                    name=f"vtmp{h}", bufs=2)
                nc.sync.dma_start_transpose(out=vtmp, in_=vT_sb[:, h, :])
                nc.vector.tensor_copy(
                    out=v_sb[:, :, h * VROW:h * VROW + HD], in_=vtmp)

            # ---- attention per head ----
            ao_sb = persist.tile([128, NT_S, E], BF16, tag="big_a")
            for h in range(H):
                pt_t = [pts.tile([128, S], BF16, tag="pt", name=f"pt{h}_{_sk}")
                        for _sk in range(NT_S)]
                for sk in range(NT_S):
                    ssp = bigps.tile([128, S], F32, tag="ps")
                    for sc in range(2):
                        nc.tensor.matmul(
                            ssp[:, sc * 512:(sc + 1) * 512],
                            kT_sb[:, h, sk * 128:(sk + 1) * 128],
                            qT_sb[:, h, sc * 512:(sc + 1) * 512],
                            start=True,
                            stop=True,
                        )
                    nc.scalar.activation(
                        out=pt_t[sk], in_=ssp, func=ACT_EXP, bias=0.0, scale=1.0
                    )
                for sq in range(NT_S):
                    op_t = bigps.tile([128, S], F32, tag="ps", name=f"op{h}_{sq}")
                    op = op_t[:, 0:VROW]
                    for sk in range(NT_S):
                        nc.tensor.matmul(
                            op,
                            pt_t[sk][:, sq * 128:(sq + 1) * 128],
                            v_sb[:, sk, h * VROW:(h + 1) * VROW],
                            start=(sk == 0),
                            stop=(sk == NT_S - 1),
                        )
                    recip = small.tile([128, 1], F32, tag="recip")
                    nc.vector.reciprocal(out=recip, in_=op[:, HD:VROW])
                    nc.vector.tensor_scalar_mul(
                        ao_sb[:, sq, h * 128:(h + 1) * 128], op[:, 0:HD], recip
                    )

            # ---- WoT resident: halves reuse the qT/kT slots (now dead) ----
            wo0_sb = persist.tile([128, NT_E, 1024], BF16, tag="qT", name="wo0")
            wo1_sb = persist.tile([128, NT_E, 1024], BF16, tag="kT", name="wo1")
            woT_r = woT.rearrange("(t p) n -> p t n", p=128)
            for et in range(NT_E):
                nc.scalar.dma_start(out=wo0_sb[:, et, :], in_=woT_r[:, et, 0:1024])
                nc.scalar.dma_start(out=wo1_sb[:, et, :], in_=woT_r[:, et, 1024:2048])
            wo_half = [wo0_sb, wo1_sb]

            # ---- transpose ao[s,e] -> aoT[e,s] ----
            aoT_sb = persist.tile([128, NT_E, S], BF16, tag="big_b")
            for sq in range(NT_S):
                nc.sync.dma_start_transpose(
                    out=aoT_sb[:, :, sq * 128:(sq + 1) * 128],
                    in_=ao_sb[:, sq, :],
                )

            # ---- output projection: out[s, e2] = aoT.T @ WoT ----
            for sg in range(2):
                for ep in range(2):  # 1024-wide e2 column halves
                    pso = [bigps.tile([128, S], F32, tag="ps", name=f"pso{_i}")
                           for _i in range(NT_S // 2)]
                    for et in range(NT_E):
                        for st4 in range(NT_S // 2):
                            st = sg * (NT_S // 2) + st4
                            for ech in range(2):
                                nc.tensor.matmul(
                                    pso[st4][:, ech * 512:(ech + 1) * 512],
                                    aoT_sb[:, et, st * 128:(st + 1) * 128],
                                    wo_half[ep][:, et, ech * 512:(ech + 1) * 512],
                                    start=(et == 0),
                                    stop=(et == NT_E - 1),
                                )
                    for st4 in range(NT_S // 2):
                        st = sg * (NT_S // 2) + st4
                        for ech in range(2):
                            ot = outsb.tile([128, 512], F32, tag="outt",
                                            name=f"ot{sg}_{ep}_{st4}_{ech}")
                            nc.vector.tensor_copy(
                                out=ot, in_=pso[st4][:, ech * 512:(ech + 1) * 512])
                            nc.sync.dma_start(
                                out=out.ap()[st * 128:(st + 1) * 128,
                                             (ep * 1024 + ech * 512):
                                             (ep * 1024 + (ech + 1) * 512)],
                                in_=ot,
                            )

    nc.compile()
    return nc


def _get_nc():
    if "nc" not in _CACHED:
        _CACHED["nc"] = _build()
    return _CACHED["nc"]


def _numpy_reference(query, key, value, attention_mask,
                     Wq, bq, Wk, bk, Wv, bv, Wo, bo):
    # general fallback (only used when attention_mask isn't all ones)
    Bb, SQ, _ = query.shape
    SK = key.shape[1]
    q = query @ Wq.T + bq
    k = key @ Wk.T + bk
    v = value @ Wv.T + bv
    q = q.reshape(Bb, SQ, H, HD).transpose(0, 2, 1, 3)
    k = k.reshape(Bb, SK, H, HD).transpose(0, 2, 1, 3)
    v = v.reshape(Bb, SK, H, HD).transpose(0, 2, 1, 3)
    scores = np.einsum("bhqd,bhkd->bhqk", q, k) * (HD ** -0.5)
    scores = np.where(attention_mask[:, None, :, :] == 0,
                      np.float32(-1e10), scores)
    scores -= scores.max(-1, keepdims=True)
    p = np.exp(scores)
    p /= p.sum(-1, keepdims=True)
    o = np.einsum("bhqk,bhkd->bhqd", p, v)
    o = o.transpose(0, 2, 1, 3).reshape(Bb, SQ, E)
    return (o @ Wo.T + bo).astype(np.float32)


def _prepare_in_maps(inputs):
    query = np.asarray(inputs["query"], dtype=np.float32)
    key = np.asarray(inputs["key"], dtype=np.float32)
    value = np.asarray(inputs["value"], dtype=np.float32)
    Wq = np.asarray(inputs["Wq"], dtype=np.float32)
    bq = np.asarray(inputs["bq"], dtype=np.float32)
    Wk = np.asarray(inputs["Wk"], dtype=np.float32)
    bk = np.asarray(inputs["bk"], dtype=np.float32)
    Wv = np.asarray(inputs["Wv"], dtype=np.float32)
    bv = np.asarray(inputs["bv"], dtype=np.float32)
    Wo = np.asarray(inputs["Wo"], dtype=np.float32)

    scale = np.float32(HD ** -0.5)
    bf = ml_dtypes.bfloat16
    wqT = np.ascontiguousarray((Wq.T * scale).astype(bf))
    wkT = np.ascontiguousarray(Wk.T.astype(bf))
    wvT = np.ascontiguousarray(Wv.T.astype(bf))
    woT = np.ascontiguousarray(Wo.T.astype(bf))
    bq_s = (bq * scale).astype(np.float32)

    in_maps = []
    for b in range(B):
        in_maps.append({
            "xqT": np.ascontiguousarray(query[b].T.astype(bf)),
            "xkT": np.ascontiguousarray(key[b].T.astype(bf)),
            "xvT": np.ascontiguousarray(value[b].T.astype(bf)),
            "wqT": wqT, "wkT": wkT, "wvT": wvT, "woT": woT,
            "bq": bq_s, "bk": bk.astype(np.float32),
            "bv": bv.astype(np.float32),
        })
    return in_maps


def run_on_device(inputs, **spmd_kwargs):
    """Run the bass kernel; returns (out [B,S,E] f32, BassKernelResults)."""
    in_maps = _prepare_in_maps(inputs)
    bo = np.asarray(inputs["bo"], dtype=np.float32)
    res = run_bass_kernel_spmd(_get_nc(), in_maps,
                               core_ids=list(range(B)), **spmd_kwargs)
    out = np.stack([res.results[b]["out"] for b in range(B)], axis=0)
    return (out + bo).astype(np.float32), res


def kernel(**inputs):
    mask = np.asarray(inputs["attention_mask"])
    if not mask.all():
        return _numpy_reference(
            np.asarray(inputs["query"], dtype=np.float32),
            np.asarray(inputs["key"], dtype=np.float32),
            np.asarray(inputs["value"], dtype=np.float32), mask,
            np.asarray(inputs["Wq"], dtype=np.float32),
            np.asarray(inputs["bq"], dtype=np.float32),
            np.asarray(inputs["Wk"], dtype=np.float32),
            np.asarray(inputs["bk"], dtype=np.float32),
            np.asarray(inputs["Wv"], dtype=np.float32),
            np.asarray(inputs["bv"], dtype=np.float32),
            np.asarray(inputs["Wo"], dtype=np.float32),
            np.asarray(inputs["bo"], dtype=np.float32))
    out, _ = run_on_device(inputs)
    return out


# revision 26
# speedup vs baseline: 1.1079x; 1.0729x over previous
"""CrossAttention kernel for 8 Trainium2 NeuronCores.

Problem (hardcoded): B=8, SQ=SK=1024, Q_DIM=2048, KV_DIM=1024, E_DIM=2048,
H=16 heads, HD=128.  out = softmax((X_q Wq^T + bq)(X_k Wk^T + bk)^T / sqrt(HD))
                            @ (X_v Wv^T + bv) @ Wo^T + bo

Sharding: data-parallel over batch — each of the 8 cores computes one batch
element end-to-end; no collectives.

Per-core dataflow (all matmuls bf16 with f32 PSUM accumulation):
  - Host pre-transposes activations/weights so every matmul sees natural
    [K_contraction on partitions] layouts:
      qT[e,s]  = (WqT tiles).T @ xqT      (e on partitions, per-head blocks)
      kT[e,s], vT[e,s] likewise; vT is DMA-transposed into v[s, e] with a
      ones column appended per head (gives softmax denominators for free).
  - Per head: scoresT[sk,sq] = kT_h.T @ qT_h ; P = exp(scoresT) (ACT, bf16)
      oP[sq, 129] = sum_sk P_tile.T @ v[sk, head||ones]   (col 128 = rowsum)
      ao[s, e] = oP[:, :128] * (1/oP[:, 128])             (per-partition)
  - ao is DMA-transposed to aoT[e, s]; out = aoT.T @ WoT (+ bo on host).
"""

import sys

sys.path.insert(0, "/opt/trn_rl_repo")

import numpy as np
import ml_dtypes

import concourse.tile as tile
from concourse import bacc
import concourse.mybir as mybir
from concourse.bass_utils import run_bass_kernel_spmd

F32 = mybir.dt.float32
BF16 = mybir.dt.bfloat16
ACT_IDENT = mybir.ActivationFunctionType.Identity
ACT_EXP = mybir.ActivationFunctionType.Exp

B = 8
S = 1024          # SQ == SK
DQ = 2048         # query input dim
DKV = 1024        # key/value input dim
E = 2048          # embed dim
H = 16            # heads
HD = 128          # head dim
NT_S = S // 128   # 8 seq tiles
NT_E = E // 128   # 16 e tiles (== heads)
NT_DQ = DQ // 128
NT_DKV = DKV // 128
VROW = HD + 1     # head block in v_sb incl. ones column

_CACHED = {}


def _build():
    nc = bacc.Bacc("TRN2", target_bir_lowering=False, debug=False)

    xqT = nc.dram_tensor("xqT", [DQ, S], BF16, kind="ExternalInput")
    xkT = nc.dram_tensor("xkT", [DKV, S], BF16, kind="ExternalInput")
    xvT = nc.dram_tensor("xvT", [DKV, S], BF16, kind="ExternalInput")
    wqT = nc.dram_tensor("wqT", [DQ, E], BF16, kind="ExternalInput")
    wkT = nc.dram_tensor("wkT", [DKV, E], BF16, kind="ExternalInput")
    wvT = nc.dram_tensor("wvT", [DKV, E], BF16, kind="ExternalInput")
    woT = nc.dram_tensor("woT", [E, E], BF16, kind="ExternalInput")
    bq = nc.dram_tensor("bq", [E], F32, kind="ExternalInput")
    bk = nc.dram_tensor("bk", [E], F32, kind="ExternalInput")
    bv = nc.dram_tensor("bv", [E], F32, kind="ExternalInput")
    out = nc.dram_tensor("out", [S, E], F32, kind="ExternalOutput")

    with tile.TileContext(nc) as tc:
        with (
            tc.tile_pool(name="persist", bufs=1) as persist,
            tc.tile_pool(name="wstream", bufs=8) as wstream,
            tc.tile_pool(name="pts", bufs=15) as pts,
            tc.tile_pool(name="outsb", bufs=4) as outsb,
            tc.tile_pool(name="small", bufs=2) as small,
            tc.tile_pool(name="bigps", bufs=4, space="PSUM") as bigps,
        ):
            # ---- load inputs / constants ----
            xq_sb = persist.tile([128, NT_DQ, S], BF16, tag="big_a")
            xk_sb = [pts.tile([128, S], BF16, tag="pt", name=f"xk{_d}")
                     for _d in range(NT_DKV)]
            xv_sb = [pts.tile([128, S], BF16, tag="pt", name=f"xv{_d}")
                     for _d in range(NT_DKV)]
            xqT_r = xqT.rearrange("(t p) s -> p t s", p=128)
            for _d in range(NT_DQ):
                nc.sync.dma_start(out=xq_sb[:, _d, :], in_=xqT_r[:, _d, :])
            xkT_r = xkT.rearrange("(t p) s -> p t s", p=128)
            xvT_r = xvT.rearrange("(t p) s -> p t s", p=128)
            for _d in range(NT_DKV):
                nc.sync.dma_start(out=xk_sb[_d], in_=xkT_r[:, _d, :])
                nc.sync.dma_start(out=xv_sb[_d], in_=xvT_r[:, _d, :])
            bq_sb = persist.tile([128, NT_E], F32, tag="bq")
            bk_sb = persist.tile([128, NT_E], F32, tag="bk")
            bv_sb = persist.tile([128, NT_E], F32, tag="bv")
            nc.sync.dma_start(out=bq_sb, in_=bq.rearrange("(t p) -> p t", p=128))
            nc.sync.dma_start(out=bk_sb, in_=bk.rearrange("(t p) -> p t", p=128))
            nc.sync.dma_start(out=bv_sb, in_=bv.rearrange("(t p) -> p t", p=128))

            qT_sb = persist.tile([128, NT_E, S], BF16, tag="qT")
            kT_sb = persist.tile([128, NT_E, S], BF16, tag="kT")
            vT_sb = persist.tile([128, NT_E, S], BF16, tag="big_b")
            v_sb = persist.tile([128, NT_S, H * VROW], BF16, tag="v")
            # ones columns for the softmax-denominator trick
            nc.vector.memset(
                v_sb.rearrange("p t (h c) -> p t h c", c=VROW)[:, :, :, HD:], 1.0
            )

            # ---- projections: produce qT/kT/vT in [e_partition, s] layout ----
            # out_psum[e128, s1024] accumulated over d tiles; weight blocks of
            # 256 e-columns streamed once; X resident.
            def project(w_dram, x_sb, nt_d, bias_sb, dst_sb, wtag):
                for eb in range(E // 256):
                    ps = [bigps.tile([128, S], F32, tag="ps", name=f"projps{_i}")
                          for _i in range(2)]
                    for d in range(nt_d):
                        wt = wstream.tile([128, 256], BF16, tag="w",
                                          name=f"w_{wtag}_{eb}_{d}")
                        nc.scalar.dma_start(
                            out=wt,
                            in_=w_dram[d * 128:(d + 1) * 128, eb * 256:(eb + 1) * 256],
                        )
                        xd = x_sb[:, d, :] if not isinstance(x_sb, list) \
                            else x_sb[d]
                        for es in range(2):
                            for sc in range(2):
                                nc.tensor.matmul(
                                    ps[es][:, sc * 512:(sc + 1) * 512],
                                    wt[:, es * 128:(es + 1) * 128],
                                    xd[:, sc * 512:(sc + 1) * 512],
                                    start=(d == 0),
                                    stop=(d == nt_d - 1),
                                )
                    for es in range(2):
                        et = eb * 2 + es
                        nc.scalar.activation(
                            out=dst_sb[:, et, :],
                            in_=ps[es],
                            func=ACT_IDENT,
                            bias=bias_sb[:, et:et + 1],
                            scale=1.0,
                        )

            project(wqT, xq_sb, NT_DQ, bq_sb, qT_sb, "wq")
            project(wkT, xk_sb, NT_DKV, bk_sb, kT_sb, "wk")
            project(wvT, xv_sb, NT_DKV, bv_sb, vT_sb, "wv")

            # vT[e,s] -> v[s, head-block] via DMA transpose (per head/e-tile)
            for h in range(H):
                vtmp = wstream.tile([128, NT_S, HD], BF16, tag="vtmp",
                                    name=f"vtmp{h}", bufs=2)
                nc.sync.dma_start_transpose(out=vtmp, in_=vT_sb[:, h, :])
                nc.vector.tensor_copy(
                    out=v_sb[:, :, h * VROW:h * VROW + HD], in_=vtmp)

            # ---- attention per head ----
            ao_sb = persist.tile([128, NT_S, E], BF16, tag="big_a")
            for h in range(H):
                pt_t = [pts.tile([128, S], BF16, tag="pt", name=f"pt{h}_{_sk}")
                        for _sk in range(NT_S)]
                for sk in range(NT_S):
                    ssp = bigps.tile([128, S], F32, tag="ps")
                    for sc in range(2):
                        nc.tensor.matmul(
                            ssp[:, sc * 512:(sc + 1) * 512],
                            kT_sb[:, h, sk * 128:(sk + 1) * 128],
                            qT_sb[:, h, sc * 512:(sc + 1) * 512],
                            start=True,
                            stop=True,
                        )
                    nc.scalar.activation(
                        out=pt_t[sk], in_=ssp, func=ACT_EXP, bias=0.0, scale=1.0
                    )
                for sq in range(NT_S):
                    op_t = bigps.tile([128, S], F32, tag="ps", name=f"op{h}_{sq}")
                    op = op_t[:, 0:VROW]
                    for sk in range(NT_S):
                        nc.tensor.matmul(
                            op,
                            pt_t[sk][:, sq * 128:(sq + 1) * 128],
                            v_sb[:, sk, h * VROW:(h + 1) * VROW],
                            start=(sk == 0),
                            stop=(sk == NT_S - 1),
                        )
                    recip = small.tile([128, 1], F32, tag="recip")
                    nc.vector.reciprocal(out=recip, in_=op[:, HD:VROW])
                    nc.vector.tensor_scalar_mul(
                        ao_sb[:, sq, h * 128:(h + 1) * 128], op[:, 0:HD], recip
                    )

            # ---- WoT resident: halves reuse the qT/kT slots (now dead) ----
            wo0_sb = persist.tile([128, NT_E, 1024], BF16, tag="qT", name="wo0")
            wo1_sb = persist.tile([128, NT_E, 1024], BF16, tag="kT", name="wo1")
            woT_r = woT.rearrange("(t p) n -> p t n", p=128)
            for et in range(NT_E):
                nc.scalar.dma_start(out=wo0_sb[:, et, :], in_=woT_r[:, et, 0:1024])
                nc.scalar.dma_start(out=wo1_sb[:, et, :], in_=woT_r[:, et, 1024:2048])
            wo_half = [wo0_sb, wo1_sb]

            # ---- transpose ao[s,e] -> aoT[e,s] ----
            aoT_sb = persist.tile([128, NT_E, S], BF16, tag="big_b")
            for sq in range(NT_S):
                nc.sync.dma_start_transpose(
                    out=aoT_sb[:, :, sq * 128:(sq + 1) * 128],
                    in_=ao_sb[:, sq, :],
                )

            # ---- output projection: out[s, e2] = aoT.T @ WoT ----
            for sg in range(2):
                for ep in range(2):  # 1024-wide e2 column halves
                    pso = [bigps.tile([128, S], F32, tag="ps", name=f"pso{_i}")
                           for _i in range(NT_S // 2)]
                    for et in range(NT_E):
                        for st4 in range(NT_S // 2):
                            st = sg * (NT_S // 2) + st4
                            for ech in range(2):
                                nc.tensor.matmul(
                                    pso[st4][:, ech * 512:(ech + 1) * 512],
                                    aoT_sb[:, et, st * 128:(st + 1) * 128],
                                    wo_half[ep][:, et, ech * 512:(ech + 1) * 512],
                                    start=(et == 0),
                                    stop=(et == NT_E - 1),
                                )
                    for st4 in range(NT_S // 2):
                        st = sg * (NT_S // 2) + st4
                        for ech in range(2):
                            ot = outsb.tile([128, 512], F32, tag="outt",
                                            name=f"ot{sg}_{ep}_{st4}_{ech}")
                            nc.vector.tensor_copy(
                                out=ot, in_=pso[st4][:, ech * 512:(ech + 1) * 512])
                            nc.sync.dma_start(
                                out=out.ap()[st * 128:(st + 1) * 128,
                                             (ep * 1024 + ech * 512):
                                             (ep * 1024 + (ech + 1) * 512)],
                                in_=ot,
                            )

    nc.compile()
    return nc


def _get_nc():
    if "nc" not in _CACHED:
        _CACHED["nc"] = _build()
    return _CACHED["nc"]


def _numpy_reference(query, key, value, attention_mask,
                     Wq, bq, Wk, bk, Wv, bv, Wo, bo):
    # general fallback (only used when attention_mask isn't all ones)
    Bb, SQ, _ = query.shape
    SK = key.shape[1]
    q = query @ Wq.T + bq
    k = key @ Wk.T + bk
    v = value @ Wv.T + bv
    q = q.reshape(Bb, SQ, H, HD).transpose(0, 2, 1, 3)
    k = k.reshape(Bb, SK, H, HD).transpose(0, 2, 1, 3)
    v = v.reshape(Bb, SK, H, HD).transpose(0, 2, 1, 3)
    scores = np.einsum("bhqd,bhkd->bhqk", q, k) * (HD ** -0.5)
    scores = np.where(attention_mask[:, None, :, :] == 0,
                      np.float32(-1e10), scores)
    scores -= scores.max(-1, keepdims=True)
    p = np.exp(scores)
    p /= p.sum(-1, keepdims=True)
    o = np.einsum("bhqk,bhkd->bhqd", p, v)
    o = o.transpose(0, 2, 1, 3).reshape(Bb, SQ, E)
    return (o @ Wo.T + bo).astype(np.float32)


def _prepare_in_maps(inputs):
    query = np.asarray(inputs["query"], dtype=np.float32)
    key = np.asarray(inputs["key"], dtype=np.float32)
    value = np.asarray(inputs["value"], dtype=np.float32)
    Wq = np.asarray(inputs["Wq"], dtype=np.float32)
    bq = np.asarray(inputs["bq"], dtype=np.float32)
    Wk = np.asarray(inputs["Wk"], dtype=np.float32)
    bk = np.asarray(inputs["bk"], dtype=np.float32)
    Wv = np.asarray(inputs["Wv"], dtype=np.float32)
    bv = np.asarray(inputs["bv"], dtype=np.float32)
    Wo = np.asarray(inputs["Wo"], dtype=np.float32)

    scale = np.float32(HD ** -0.5)
    bf = ml_dtypes.bfloat16
    wqT = np.ascontiguousarray((Wq.T * scale).astype(bf))
    wkT = np.ascontiguousarray(Wk.T.astype(bf))
    wvT = np.ascontiguousarray(Wv.T.astype(bf))
    woT = np.ascontiguousarray(Wo.T.astype(bf))
    bq_s = (bq * scale).astype(np.float32)

    in_maps = []
    for b in range(B):
        in_maps.append({
            "xqT": np.ascontiguousarray(query[b].T.astype(bf)),
            "xkT": np.ascontiguousarray(key[b].T.astype(bf)),
            "xvT": np.ascontiguousarray(value[b].T.astype(bf)),
            "wqT": wqT, "wkT": wkT, "wvT": wvT, "woT": woT,
            "bq": bq_s, "bk": bk.astype(np.float32),
            "bv": bv.astype(np.float32),
        })
    return in_maps


def run_on_device(inputs, **spmd_kwargs):
    """Run the bass kernel; returns (out [B,S,E] f32, BassKernelResults)."""
    in_maps = _prepare_in_maps(inputs)
    bo = np.asarray(inputs["bo"], dtype=np.float32)
    res = run_bass_kernel_spmd(_get_nc(), in_maps,
                               core_ids=list(range(B)), **spmd_kwargs)
    out = np.stack([res.results[b]["out"] for b in range(B)], axis=0)
    return (out + bo).astype(np.float32), res


def kernel(**inputs):
    mask = np.asarray(inputs["attention_mask"])
    if not mask.all():
        return _numpy_reference(
            np.asarray(inputs["query"], dtype=np.float32),
            np.asarray(inputs["key"], dtype=np.float32),
            np.asarray(inputs["value"], dtype=np.float32), mask,
            np.asarray(inputs["Wq"], dtype=np.float32),
            np.asarray(inputs["bq"], dtype=np.float32),
            np.asarray(inputs["Wk"], dtype=np.float32),
            np.asarray(inputs["bk"], dtype=np.float32),
            np.asarray(inputs["Wv"], dtype=np.float32),
            np.asarray(inputs["bv"], dtype=np.float32),
            np.asarray(inputs["Wo"], dtype=np.float32),
            np.asarray(inputs["bo"], dtype=np.float32))
    out, _ = run_on_device(inputs)
    return out
